# revision 11
# baseline (speedup 1.0000x reference)
"""Trainium2 Bass kernel for MAF_Extractor (projection + bilinear grid_sample + 1x1-conv MLP).

Strategy (8 NeuronCores, pure data parallel over batch B=64 -> 8 batches/core):
  - Host: transpose s_feat to channels-last [B, H*W, C] bf16 so one sampled pixel's
    256 channels are contiguous (1 gather row = 512B); pack per-point data in the
    two on-chip layouts the kernel needs; pack MLP weights as lhsT blocks.
  - Device: compute bilinear indices/weights on DVE, gather 4 corners per point with
    dma_gather(transpose=True) landing directly in [channel, point] layout, weight +
    accumulate on DVE, then the 3-layer MLP on the PE with fused bias/leaky-relu
    epilogues, and a final strided DMA to the [8, 2155] output shard.

Numerics: s_feat/gather/MLP inputs in bf16, all matmul accumulation in fp32 PSUM.
Measured absmax error vs the fp32 reference ~3e-3 on output scale ~1.2.
"""

import numpy as np
import ml_dtypes

import concourse.bass as bass
import concourse.bacc as bacc
import concourse.mybir as mybir
import concourse.tile as tile
from concourse.bass_utils import run_bass_kernel_spmd

F32 = mybir.dt.float32
BF16 = mybir.dt.bfloat16
I16 = mybir.dt.int16
AL = mybir.AluOpType

# Problem constants (hardcoded per harness contract)
B, N, C, H, W = 64, 431, 256, 56, 56
NCORES = 8
BPC = B // NCORES            # batches per core
NPTS = BPC * N               # 3448 real points per core
NPAD = 3456                  # padded to 128*27
NCOLS = NPAD // 128          # 27  (q-major layout [128, 27], q = p*27 + j)
WCOLS = NPAD // 16           # 216 (wrapped layout [16, 216], i = cc*16 + pp)
CHUNKS = 4
QC = NPAD // CHUNKS          # 864 points per chunk
WCC = QC // 16               # 54 wrapped cols per (chunk, corner)
MLP_SUB = 2
NSUB = QC // MLP_SUB         # 432 columns per matmul chunk
HW_PIX = H * W               # 3136
ROWS = BPC * HW_PIX          # 25088 gather rows per core
PAD_VAL = 1.0e9              # pad points project far out of range -> zero weights

_CACHE = {}


def _build_nc():
    nc = bacc.Bacc("TRN2", target_bir_lowering=False)

    feat = nc.dram_tensor("feat", [ROWS, C], BF16, kind="ExternalInput")
    ptw = nc.dram_tensor("ptw", [16, 6 * WCOLS], F32, kind="ExternalInput")
    ptq = nc.dram_tensor("ptq", [128, 5 * NCOLS], F32, kind="ExternalInput")
    w0t = nc.dram_tensor("w0t", [128, 256], BF16, kind="ExternalInput")
    w1t = nc.dram_tensor("w1t", [128, 192], BF16, kind="ExternalInput")
    w2t = nc.dram_tensor("w2t", [128, 15], BF16, kind="ExternalInput")
    bias = nc.dram_tensor("bias", [128, 3], F32, kind="ExternalInput")
    out = nc.dram_tensor("out", [BPC, 5 * N], F32, kind="ExternalOutput")

    with tile.TileContext(nc) as tc:
        _emit(tc, nc, feat, ptw, ptq, w0t, w1t, w2t, bias, out)
    nc.compile()
    return nc


def _coord_pipeline(v, pool, P, FD, px, py, s, tx, ty, tag):
    """Shared projection+floor pipeline on [P, FD] tiles.

    Returns dict with xc (clamped x), fx (frac), x0f (floor), and y equivalents.
    """
    res = {}
    for name, pc, tc_ in (("x", px, tx), ("y", py, ty)):
        t1 = pool.tile([P, FD], F32, tag=f"{tag}{name}t1")
        v.tensor_tensor(t1[:], pc, tc_, AL.add)
        t2 = pool.tile([P, FD], F32, tag=f"{tag}{name}t2")
        v.tensor_tensor(t2[:], t1[:], s, AL.mult)
        xr = pool.tile([P, FD], F32, tag=f"{tag}{name}r")
        # x = (p2d + 1) * 0.5 * (W-1) = 27.5*p2d + 27.5
        v.tensor_scalar(xr[:], t2[:], 27.5, 27.5, AL.mult, AL.add)
        xc = pool.tile([P, FD], F32, tag=f"{tag}{name}c")
        v.tensor_scalar(xc[:], xr[:], -4.0, 60.0, AL.max, AL.min)
        # floor via int32 conversion + compare-correct; exact whether the
        # HW converter truncates (sim) or rounds to nearest (hardware)
        xs = pool.tile([P, FD], F32, tag=f"{tag}{name}s")
        v.tensor_scalar(xs[:], xc[:], 4.0, None, AL.add)
        xi = pool.tile([P, FD], mybir.dt.int32, tag=f"{tag}{name}i")
        v.tensor_copy(xi[:], xs[:])
        xf = pool.tile([P, FD], F32, tag=f"{tag}{name}g")
        v.tensor_copy(xf[:], xi[:])
        gt = pool.tile([P, FD], F32, tag=f"{tag}{name}gt")
        v.tensor_tensor(gt[:], xf[:], xs[:], AL.is_gt)
        x0 = pool.tile([P, FD], F32, tag=f"{tag}{name}0")
        v.tensor_tensor(x0[:], xf[:], gt[:], AL.subtract)
        v.tensor_scalar(x0[:], x0[:], -4.0, None, AL.add)
        fx = pool.tile([P, FD], F32, tag=f"{tag}{name}f")
        v.tensor_tensor(fx[:], xc[:], x0[:], AL.subtract)
        res[name] = (xc, fx, x0)
    return res


def _emit(tc, nc, feat, ptw, ptq, w0t, w1t, w2t, bias, out):
    v = nc.vector
    import contextlib

    ctx = contextlib.ExitStack()
    pool = ctx.enter_context(tc.tile_pool(name="main", bufs=1))
    gpool = ctx.enter_context(tc.tile_pool(name="gather", bufs=2))
    psum = ctx.enter_context(tc.tile_pool(name="psum", bufs=2, space="PSUM"))

    # ---- load inputs ----
    ptw_t = pool.tile([16, 6 * WCOLS], F32)
    nc.sync.dma_start(ptw_t[:], ptw[:])
    ptq_t = pool.tile([128, 5 * NCOLS], F32)
    nc.sync.dma_start(ptq_t[:], ptq[:])
    w0t_t = pool.tile([128, 256], BF16)
    nc.sync.dma_start(w0t_t[:], w0t[:])
    w1t_t = pool.tile([128, 192], BF16)
    nc.sync.dma_start(w1t_t[:], w1t[:])
    w2t_t = pool.tile([128, 15], BF16)
    nc.sync.dma_start(w2t_t[:], w2t[:])
    bias_t = pool.tile([128, 3], F32)
    nc.sync.dma_start(bias_t[:], bias[:])

    WC = WCOLS

    def wf(i):  # wrapped field slice
        return ptw_t[:, i * WC:(i + 1) * WC]

    # ---- wrapped-layout index pipeline ([16, 216]) ----
    cw = _coord_pipeline(v, pool, 16, WC, wf(0), wf(1), wf(2), wf(3), wf(4), "w")
    basep = wf(5)
    x0c = pool.tile([16, WC], F32)
    v.tensor_scalar(x0c[:], cw["x"][2][:], 0.0, 55.0, AL.max, AL.min)
    x1c = pool.tile([16, WC], F32)
    v.tensor_scalar(x1c[:], cw["x"][2][:], 1.0, 0.0, AL.add, AL.max)
    v.tensor_scalar(x1c[:], x1c[:], 55.0, None, AL.min)
    y0c = pool.tile([16, WC], F32)
    v.tensor_scalar(y0c[:], cw["y"][2][:], 0.0, 55.0, AL.max, AL.min)
    y1c = pool.tile([16, WC], F32)
    v.tensor_scalar(y1c[:], cw["y"][2][:], 1.0, 0.0, AL.add, AL.max)
    v.tensor_scalar(y1c[:], y1c[:], 55.0, None, AL.min)

    row0 = pool.tile([16, WC], F32)
    v.scalar_tensor_tensor(row0[:], y0c[:], 56.0, basep, AL.mult, AL.add)
    row1 = pool.tile([16, WC], F32)
    v.scalar_tensor_tensor(row1[:], y1c[:], 56.0, basep, AL.mult, AL.add)

    idxf = pool.tile([16, 4 * WC], F32)
    v.tensor_tensor(idxf[:, 0 * WC:1 * WC], row0[:], x0c[:], AL.add)
    v.tensor_tensor(idxf[:, 1 * WC:2 * WC], row0[:], x1c[:], AL.add)
    v.tensor_tensor(idxf[:, 2 * WC:3 * WC], row1[:], x0c[:], AL.add)
    v.tensor_tensor(idxf[:, 3 * WC:4 * WC], row1[:], x1c[:], AL.add)

    # int16 gather index table, ordered i = chunk*(4*QC) + corner*QC + qq:
    # wrapped col block for (chunk c, corner k) = c*216 + k*54, sourced from
    # corner k's f32 index cols [c*54, (c+1)*54).
    idx_t = pool.tile([128, 4 * WC], I16)
    for c in range(CHUNKS):
        for k in range(4):
            v.tensor_copy(
                idx_t[0:16, c * 216 + k * WCC:c * 216 + (k + 1) * WCC],
                idxf[:, k * WC + c * WCC:k * WC + (c + 1) * WCC],
            )
    # replicate idx rows 0:16 across all 128 partitions (gather ucode reads
    # a copy per 16-partition group)
    nc.sync.dma_start(idx_t[16:32, :], idx_t[0:16, :])
    nc.sync.dma_start(idx_t[32:64, :], idx_t[0:32, :])
    nc.sync.dma_start(idx_t[64:128, :], idx_t[0:64, :])

    # ---- q-major weight pipeline ([128, 27]) ----
    NQ = NCOLS

    def qf(i):
        return ptq_t[:, i * NQ:(i + 1) * NQ]

    cq = _coord_pipeline(v, pool, 128, NQ, qf(0), qf(1), qf(2), qf(3), qf(4), "q")

    def edge_weights(x0, fx, tagp):
        # a0 = (1-fx)*[0<=x0<=55], a1 = fx*[0<=x0+1<=55]
        w0 = pool.tile([128, NQ], F32, tag=f"{tagp}w0")
        v.tensor_scalar(w0[:], fx[:], -1.0, 1.0, AL.mult, AL.add)
        va = pool.tile([128, NQ], F32, tag=f"{tagp}va")
        v.tensor_scalar(va[:], x0[:], 0.0, None, AL.is_ge)
        vb = pool.tile([128, NQ], F32, tag=f"{tagp}vb")
        v.tensor_scalar(vb[:], x0[:], 55.0, None, AL.is_le)
        v0 = pool.tile([128, NQ], F32, tag=f"{tagp}v0")
        v.tensor_tensor(v0[:], va[:], vb[:], AL.mult)
        a0 = pool.tile([128, NQ], F32, tag=f"{tagp}a0")
        v.tensor_tensor(a0[:], w0[:], v0[:], AL.mult)
        # validity of x0+1: x0 >= -1 and x0 <= 54
        v.tensor_scalar(va[:], x0[:], -1.0, None, AL.is_ge)
        v.tensor_scalar(vb[:], x0[:], 54.0, None, AL.is_le)
        v.tensor_tensor(v0[:], va[:], vb[:], AL.mult)
        a1 = pool.tile([128, NQ], F32, tag=f"{tagp}a1")
        v.tensor_tensor(a1[:], fx[:], v0[:], AL.mult)
        return a0, a1

    ax0, ax1 = edge_weights(cq["x"][2], cq["x"][1], "ex")
    by0, by1 = edge_weights(cq["y"][2], cq["y"][1], "ey")

    wb_full = []
    for k, (ax, by) in enumerate(((ax0, by0), (ax1, by0), (ax0, by1), (ax1, by1))):
        cw_ = pool.tile([128, NQ], F32, tag="cw")
        v.tensor_tensor(cw_[:], ax[:], by[:], AL.mult)
        cbf = pool.tile([128, NQ], BF16, tag=f"cbf{k}")
        v.tensor_copy(cbf[:], cw_[:])
        wflat = pool.tile([1, NPAD], BF16, tag=f"wfl{k}")
        nc.sync.dma_start(wflat[:], cbf[:])
        wbk = pool.tile([128, NPAD], BF16, tag=f"wb{k}")
        nc.gpsimd.partition_broadcast(wbk[:], wflat[:])
        wb_full.append(wbk)

    # ---- chunked gather + weighting + MLP ----
    pf = pool.tile([128, 2 * NPAD], BF16)   # [c_half, q] point features
    y2f = pool.tile([5, NPAD], F32)

    for c in range(CHUNKS):
        g = gpool.tile([128, 2 * 4 * QC], BF16, tag="g")
        nc.gpsimd.dma_gather(
            out_ap=g[:].rearrange("p (a b) -> p a b", a=2),
            in_ap=feat[:],
            idxs_ap=idx_t[:, c * 216:(c + 1) * 216],
            num_idxs=4 * QC,
            num_idxs_reg=4 * QC,
            elem_size=C,
            transpose=True,
            single_packet=False,
        )
        # pf[:, j2, c*QC : (c+1)*QC] = sum_k wb_k * g[:, j2, k]
        for j2 in range(2):
            dst = pf[:, j2 * NPAD + c * QC:j2 * NPAD + (c + 1) * QC]
            tmp = pool.tile([128, QC], BF16, tag="wtmp")
            for k in range(4):
                gk = g[:, j2 * 4 * QC + k * QC:j2 * 4 * QC + (k + 1) * QC]
                wk = wb_full[k][:, c * QC:(c + 1) * QC]
                if k == 0:
                    v.tensor_tensor(dst, gk, wk, AL.mult)
                else:
                    v.tensor_tensor(tmp[:], gk, wk, AL.mult)
                    v.tensor_tensor(dst, dst, tmp[:], AL.add)

        for s_ in range(MLP_SUB):
            n0 = c * QC + s_ * NSUB
            nn = NSUB
            pf0 = pf[:, 0 * NPAD + n0:0 * NPAD + n0 + nn]
            pf1 = pf[:, 1 * NPAD + n0:1 * NPAD + n0 + nn]

            ps0 = psum.tile([128, NSUB], F32, tag="ps0")
            nc.tensor.matmul(out=ps0[:], lhsT=w0t_t[:, 0:128], rhs=pf0,
                             start=True, stop=False)
            nc.tensor.matmul(out=ps0[:], lhsT=w0t_t[:, 128:256], rhs=pf1,
                             start=False, stop=True)
            pre0 = pool.tile([128, NSUB], BF16, tag="pre0")
            v.tensor_scalar(pre0[:], ps0[:], bias_t[:, 0:1], None, AL.add)
            y0t = pool.tile([128, NSUB], BF16, tag="y0t")
            v.scalar_tensor_tensor(y0t[:], pre0[:], 0.01, pre0[:], AL.mult, AL.max)

            ps1 = psum.tile([64, NSUB], F32, tag="ps1")
            nc.tensor.matmul(out=ps1[:], lhsT=w1t_t[:, 0:64], rhs=y0t[:],
                             start=True, stop=False)
            nc.tensor.matmul(out=ps1[:], lhsT=w1t_t[:, 64:128], rhs=pf0,
                             start=False, stop=False)
            nc.tensor.matmul(out=ps1[:], lhsT=w1t_t[:, 128:192], rhs=pf1,
                             start=False, stop=True)
            pre1 = pool.tile([64, NSUB], BF16, tag="pre1")
            v.tensor_scalar(pre1[:], ps1[:], bias_t[0:64, 1:2], None, AL.add)
            y1t = pool.tile([64, NSUB], BF16, tag="y1t")
            v.scalar_tensor_tensor(y1t[:], pre1[:], 0.01, pre1[:], AL.mult, AL.max)

            ps2 = psum.tile([5, NSUB], F32, tag="ps2")
            nc.tensor.matmul(out=ps2[:], lhsT=w2t_t[0:64, 0:5], rhs=y1t[:],
                             start=True, stop=False)
            nc.tensor.matmul(out=ps2[:], lhsT=w2t_t[:, 5:10], rhs=pf0,
                             start=False, stop=False)
            nc.tensor.matmul(out=ps2[:], lhsT=w2t_t[:, 10:15], rhs=pf1,
                             start=False, stop=True)
            v.tensor_scalar(y2f[0:5, n0:n0 + nn], ps2[:], bias_t[0:5, 2:3], 0.0,
                            AL.add, AL.max)

    # ---- output: out[b, c*431+n] = y2f[c, b*431+n] ----
    nc.sync.dma_start(
        out[:, :].rearrange("b (c n) -> c b n", n=N),
        y2f[:, 0:NPTS].rearrange("c (b n) -> c b n", n=N),
    )
    ctx.close()


def _host_prep(p, cam, s_feat, W0, b0, W1, b1, W2, b2):
    """Build per-core input maps. All heavy layout work is plain numpy."""
    bf = ml_dtypes.bfloat16
    # channels-last bf16 feature rows: [B, H*W, C]
    feat_t = np.ascontiguousarray(
        s_feat.transpose(0, 2, 3, 1).reshape(B, HW_PIX, C).astype(bf))

    # shared weight packs
    def pack_lhsT(Wm, kblocks):
        # Wm [M, K] -> lhsT [K, M] -> [128, kblocks*M] with kb-major cols
        Wt = Wm.T  # [K, M]
        K, M = Wt.shape
        out_ = np.zeros((128, kblocks * M), np.float32)
        for kb in range(kblocks):
            blk = Wt[kb * 128:(kb + 1) * 128]
            out_[:blk.shape[0], kb * M:kb * M + M] = blk
        return out_.astype(bf)

    w0t = pack_lhsT(W0, 2)                      # [128, 256]
    w1t = pack_lhsT(W1, 3)                      # [128, 192]
    # W2: K=320 = 64(y1) + 2*128(pf)
    W2t = W2.T  # [320, 5]
    w2t = np.zeros((128, 15), np.float32)
    w2t[0:64, 0:5] = W2t[0:64]
    w2t[:, 5:10] = W2t[64:192]
    w2t[:, 10:15] = W2t[192:320]
    w2t = w2t.astype(bf)

    bias = np.zeros((128, 3), np.float32)
    bias[:128, 0] = b0
    bias[0:64, 1] = b1
    bias[0:5, 2] = b2

    in_maps = []
    for m in range(NCORES):
        bs = slice(m * BPC, (m + 1) * BPC)
        pp = p[bs]          # [8, 431, 3]
        cc = cam[bs]        # [8, 3]

        fld = np.full((6, NPAD), 0.0, np.float32)
        fld[0, :NPTS] = pp[..., 0].reshape(-1)
        fld[1, :NPTS] = pp[..., 1].reshape(-1)
        fld[0, NPTS:] = PAD_VAL
        fld[1, NPTS:] = PAD_VAL
        brep = np.repeat(np.arange(BPC), N)
        fld[2, :NPTS] = cc[brep, 0]
        fld[3, :NPTS] = cc[brep, 1]
        fld[4, :NPTS] = cc[brep, 2]
        fld[2, NPTS:] = 1.0
        fld[5, :NPTS] = brep * float(HW_PIX)

        # wrapped [16, 216]: value for i = cc*16 + pp at (pp, cc)
        ptw = np.ascontiguousarray(
            fld.reshape(6, WCOLS, 16).transpose(2, 0, 1).reshape(16, 6 * WCOLS))
        # q-major [128, 27]: value for q = p*27 + j at (p, j)
        ptq = np.ascontiguousarray(
            fld[:5].reshape(5, 128, NCOLS).transpose(1, 0, 2).reshape(128, 5 * NCOLS))

        in_maps.append({
            "feat": feat_t[bs].reshape(ROWS, C),
            "ptw": ptw,
            "ptq": ptq,
            "w0t": w0t,
            "w1t": w1t,
            "w2t": w2t,
            "bias": bias,
        })
    return in_maps


def kernel(**inputs) -> np.ndarray:
    p = np.asarray(inputs["p"], np.float32)
    cam = np.asarray(inputs["cam"], np.float32)
    s_feat = np.asarray(inputs["s_feat"], np.float32)
    W0 = np.asarray(inputs["W0"], np.float32)
    b0 = np.asarray(inputs["b0"], np.float32)
    W1 = np.asarray(inputs["W1"], np.float32)
    b1 = np.asarray(inputs["b1"], np.float32)
    W2 = np.asarray(inputs["W2"], np.float32)
    b2 = np.asarray(inputs["b2"], np.float32)

    if "nc" not in _CACHE:
        _CACHE["nc"] = _build_nc()
    nc = _CACHE["nc"]

    in_maps = _host_prep(p, cam, s_feat, W0, b0, W1, b1, W2, b2)
    res = run_bass_kernel_spmd(nc, in_maps, core_ids=list(range(NCORES)))
    out = np.concatenate([r["out"] for r in res.results], axis=0)
    return out.astype(np.float32)
